# revision 9
# baseline (speedup 1.0000x reference)
"""Trainium2 Bass kernel for blockwise 8x8 DCT feature extraction.

For x of shape (4, 3, 64, 224, 224):
  grayscale -> per-frame 8x8-block 2D DCT-II (norm=None) -> zigzag order
  -> drop DC + last `remove_last_ac` AC coeffs -> (4, 64, 784*S),
  S = 63 - remove_last_ac.

Sharding: batch*time across 8 cores (core k: b = k//2, t-half = k%2),
no cross-core communication.

Per-core dataflow (32 frames, in octets of 8):
  1. One contiguous ~1.6MB DMA per channel per octet into
     [112 partitions = (f4, i), 3584 free = (c2, m, w)]   (h = i*8 + m,
     c2 = which chunk-of-4-frames, f4 = frame-in-chunk, i = block row).
  2. Grayscale via two fused scalar_tensor_tensor ops (DVE handles one
     half, GPSIMD the other); the third channel weight is folded into the
     DCT matmul weights.
  3. PE transposes: stationary = contiguous [112, 32] slices of the gray
     tile (fixed m, w-run = (jl, n) for a quad of blocks jq), moving =
     identity -> PSUM [32-strip at m4*32, 112] via column tiling.  The
     128-partition PSUM space is (m4, jl, n); two tiles (A: m 0..3,
     B: m 4..7).
  4. Kron DCT matmuls: for each block-quad jq, two accumulating matmuls
     lhsT = T2{A,B}[:, jq slice] [128, 112], rhs = Wp{A,B} [128, 4*S]
     with Wp[(m4, jl, n), (jl', s)] = delta(jl==jl') * D[k_s, m] *
     D[l_s, n] * GRAY[2].  Output [(f4, i), (jl, s)] is exactly the
     required DRAM order.
  5. One contiguous ~1.2MB store per octet.
"""

import numpy as np
from contextlib import ExitStack

import concourse.bass as bass
import concourse.mybir as mybir
from concourse import bacc, tile
from concourse.bass_utils import run_bass_kernel_spmd

_GRAY = np.array([0.2989, 0.587, 0.114], dtype=np.float32)
_BLK = 8
_B, _C, _T, _H, _W = 4, 3, 64, 224, 224
_NCORES = 8
_FPC = _T * _B // _NCORES  # 32 frames per core
_NI = _H // _BLK  # 28 block rows
_NJ = _W // _BLK  # 28 block cols
_NJQ = _NJ // 4  # 7 block quads
_F4 = 4  # frames per chunk
_P = _F4 * _NI  # 112 partitions
_FPO = 8  # frames per DMA octet
_NOCT = _FPC // _FPO  # 4 octets per core

_f32 = mybir.dt.float32
_GP_FD = 1600  # free-dim share of grayscale handled by GPSIMD (of 3584)


def _zigzag_idx(n=_BLK):
    idx = []
    for diag in range(2 * n - 1):
        if diag % 2 == 0:
            row = min(diag, n - 1); col = diag - row
            while row >= 0 and col < n:
                idx.append(row * n + col); row -= 1; col += 1
        else:
            col = min(diag, n - 1); row = diag - col
            while col >= 0 and row < n:
                idx.append(row * n + col); row += 1; col -= 1
    return np.array(idx, dtype=np.int64)


def _dct_mat(N=_BLK):
    n = np.arange(N, dtype=np.float64)
    k = np.arange(N, dtype=np.float64)[:, None]
    return 2.0 * np.cos(np.pi * (2.0 * n + 1.0) * k / (2.0 * N))


def _build_consts(S):
    keep = _zigzag_idx()[1:1 + S]
    D = _dct_mat()
    # Wp{A,B}[(m4, jl, n), jl*S + s] = D[k_s, m] * D[l_s, n] * GRAY[2]
    WpA = np.zeros((128, 4 * S), dtype=np.float32)
    WpB = np.zeros((128, 4 * S), dtype=np.float32)
    for m4 in range(4):
        for jl in range(4):
            for n in range(_BLK):
                q = m4 * 32 + jl * 8 + n
                for s in range(S):
                    k, l = divmod(int(keep[s]), _BLK)
                    col = jl * S + s
                    WpA[q, col] = np.float32(D[k, m4] * D[l, n] * _GRAY[2])
                    WpB[q, col] = np.float32(D[k, m4 + 4] * D[l, n] * _GRAY[2])
    eye = np.eye(_P, dtype=np.float32)
    return WpA, WpB, eye


def build_nc(S, n_oct=_NOCT, n_rep=1):
    """Build and compile the per-core Bass program (n_oct * 8 frames).

    n_rep > 1 wraps the whole pipeline in a hardware loop that recomputes
    the same result n_rep times — used only for wall-clock timing.
    """
    WpA_np, WpB_np, eye_np = _build_consts(S)
    fpc = n_oct * _FPO
    S4 = 4 * S
    qlen = _NJ * S  # free extent of one chunk4 in the out tile

    w0 = float(_GRAY[0] / _GRAY[2])
    w1 = float(_GRAY[1] / _GRAY[2])

    nc = bacc.Bacc("TRN2", target_bir_lowering=False, debug=False)
    x_d = nc.dram_tensor("x", [_C, fpc, _H, _W], _f32, kind="ExternalInput")
    o_d = nc.dram_tensor("out", [fpc, _NI * _NJ * S], _f32, kind="ExternalOutput")
    I_d = nc.inline_tensor(eye_np, "I_const")
    WpA_d = nc.inline_tensor(WpA_np, "WpA_const")
    WpB_d = nc.inline_tensor(WpB_np, "WpB_const")

    t2_groups = [(0, 4), (4, 7)]
    kron_groups = [(0, 2), (2, 4), (4, 6), (6, 7)]

    ncopy = 0

    def copy_evict(dst, src):
        nonlocal ncopy
        if ncopy % 3 == 1:
            nc.vector.tensor_copy(dst, src)
        else:
            nc.scalar.copy(dst, src)
        ncopy += 1

    mult = mybir.AluOpType.mult
    add = mybir.AluOpType.add

    with tile.TileContext(nc) as tc, ExitStack() as ctx:
        cpool = ctx.enter_context(tc.tile_pool(name="const", bufs=1))
        xpool = ctx.enter_context(tc.tile_pool(name="xin", bufs=2))
        gpool = ctx.enter_context(tc.tile_pool(name="gray", bufs=2))
        t2pool = ctx.enter_context(tc.tile_pool(name="t2", bufs=2))
        opool = ctx.enter_context(tc.tile_pool(name="outp", bufs=2))
        pspool = ctx.enter_context(
            tc.tile_pool(name="ps", bufs=2, space=bass.MemorySpace.PSUM))

        I_sb = cpool.tile([_P, _P], _f32)
        WpA_sb = cpool.tile([128, S4], _f32)
        WpB_sb = cpool.tile([128, S4], _f32)
        nc.sync.dma_start(I_sb[:], I_d.ap())
        nc.sync.dma_start(WpA_sb[:], WpA_d.ap())
        nc.sync.dma_start(WpB_sb[:], WpB_d.ap())

        xap = x_d.ap()
        oap = o_d.ap()

        rep = ctx.enter_context(tc.For_i(0, n_rep, 1)) if n_rep > 1 else None
        del rep
        for o in range(n_oct):
            X = []
            for c in range(_C):
                xt = xpool.tile([_P, 2 * 1792], _f32, tag=f"x{c}")
                src = xap[c, o * _FPO:(o + 1) * _FPO].rearrange(
                    "(c2 f4) (i m) w -> (f4 i) c2 m w", c2=2, f4=_F4, m=_BLK)
                dst = xt.rearrange("p (c2 m w) -> p c2 m w", c2=2, m=_BLK)
                nc.sync.dma_start(dst, src)
                X.append(xt)
            # grayscale (third weight folded into Wp); DVE takes the fused
            # 2-op path on [0:dv], GPSIMD the 4-op path on [dv:3584]
            g8 = gpool.tile([_P, 2 * 1792], _f32, tag="g8")
            dv = 3584 - _GP_FD
            sl = slice(0, dv)
            nc.vector.scalar_tensor_tensor(
                g8[:, sl], X[0][:, sl], w0, X[2][:, sl], mult, add)
            nc.vector.scalar_tensor_tensor(
                g8[:, sl], X[1][:, sl], w1, g8[:, sl], mult, add)
            if _GP_FD:
                sl = slice(dv, 3584)
                tg = gpool.tile([_P, _GP_FD], _f32, tag="gt")
                ug = gpool.tile([_P, _GP_FD], _f32, tag="gu")
                nc.gpsimd.tensor_scalar_mul(tg[:], X[0][:, sl], w0)
                nc.gpsimd.tensor_tensor(tg[:], tg[:], X[2][:, sl], add)
                nc.gpsimd.tensor_scalar_mul(ug[:], X[1][:, sl], w1)
                nc.gpsimd.tensor_tensor(g8[:, sl], tg[:], ug[:], add)
            gv = g8.rearrange("p (c2 m w) -> p c2 m w", c2=2, m=_BLK)

            OUT8 = opool.tile([_P, 2 * qlen], _f32, tag="o8")
            for half in range(2):
                T2A = t2pool.tile([128, _NJQ * _P], _f32, tag="t2a")
                T2B = t2pool.tile([128, _NJQ * _P], _f32, tag="t2b")
                for (g0, g1) in t2_groups:
                    psA = pspool.tile([128, (g1 - g0) * _P], _f32, tag="psA")
                    psB = pspool.tile([128, (g1 - g0) * _P], _f32, tag="psB")
                    for jj, jq in enumerate(range(g0, g1)):
                        for m in range(_BLK):
                            mh, m4 = divmod(m, 4)
                            ps = psA if mh == 0 else psB
                            nc.tensor.matmul(
                                ps[m4 * 32:(m4 + 1) * 32,
                                   jj * _P:(jj + 1) * _P],
                                gv[:, half, m, jq * 32:(jq + 1) * 32],
                                I_sb[:],
                                start=True, stop=True,
                                tile_position=(0, m4 * 32))
                    copy_evict(T2A[:, g0 * _P:g1 * _P], psA[:])
                    copy_evict(T2B[:, g0 * _P:g1 * _P], psB[:])
                for (k0, k1) in kron_groups:
                    Ops = pspool.tile([_P, (k1 - k0) * S4], _f32, tag="ops")
                    for jj, jq in enumerate(range(k0, k1)):
                        nc.tensor.matmul(
                            Ops[:, jj * S4:(jj + 1) * S4],
                            T2A[:, jq * _P:(jq + 1) * _P],
                            WpA_sb[:],
                            start=True, stop=False)
                        nc.tensor.matmul(
                            Ops[:, jj * S4:(jj + 1) * S4],
                            T2B[:, jq * _P:(jq + 1) * _P],
                            WpB_sb[:],
                            start=False, stop=True)
                    copy_evict(
                        OUT8[:, half * qlen + k0 * S4: half * qlen + k1 * S4],
                        Ops[:])
            dsto = oap[o * _FPO:(o + 1) * _FPO].rearrange(
                "(c2 f4) (i q) -> (f4 i) c2 q", c2=2, i=_NI)
            nc.sync.dma_start(dsto, OUT8.rearrange("p (c2 q) -> p c2 q", c2=2))

    nc.compile()
    return nc


def _shard_inputs(x):
    in_maps = []
    for k in range(_NCORES):
        b, th = k // 2, k % 2
        xs = np.ascontiguousarray(
            x[b, :, th * _FPC:(th + 1) * _FPC], dtype=np.float32)
        in_maps.append({"x": xs})
    return in_maps


def kernel(x, remove_last_ac):
    x = np.asarray(x)
    r = int(remove_last_ac)
    S = _BLK * _BLK - 1 - r
    nc = build_nc(S)
    res = run_bass_kernel_spmd(nc, _shard_inputs(x), list(range(_NCORES)))
    out = np.empty((_B, _T, _NI * _NJ * S), dtype=np.float32)
    for k in range(_NCORES):
        b, th = k // 2, k % 2
        out[b, th * _FPC:(th + 1) * _FPC] = res.results[k]["out"]
    return out


# revision 32
# speedup vs baseline: 1.7940x; 1.7940x over previous
"""Trainium2 Bass kernel for blockwise 8x8 DCT feature extraction.

For x of shape (4, 3, 64, 224, 224):
  grayscale -> per-frame 8x8-block 2D DCT-II (norm=None) -> zigzag order
  -> drop DC + last `remove_last_ac` AC coeffs -> (4, 64, 784*S),
  S = 63 - remove_last_ac.

Sharding: batch*time across 8 cores (core k: b = k//2, t-half = k%2),
no cross-core communication.

Per-core dataflow (32 frames, in octets of 8):
  1. One contiguous ~1.6MB DMA per channel per octet into
     [112 partitions = (f4, i), 3584 free = (c2, m, w)]   (h = i*8 + m,
     c2 = which chunk-of-4-frames, f4 = frame-in-chunk, i = block row).
  2. Grayscale via two fused scalar_tensor_tensor ops (DVE handles one
     half, GPSIMD the other); the third channel weight is folded into the
     DCT matmul weights.
  3. PE transposes: stationary = contiguous [112, 32] slices of the gray
     tile (fixed m, w-run = (jl, n) for a quad of blocks jq), moving =
     identity -> PSUM [32-strip at m4*32, 112] via column tiling.  The
     128-partition PSUM space is (m4, jl, n); two tiles (A: m 0..3,
     B: m 4..7).
  4. Kron DCT matmuls: for each block-quad jq, two accumulating matmuls
     lhsT = T2{A,B}[:, jq slice] [128, 112], rhs = Wp{A,B} [128, 4*S]
     with Wp[(m4, jl, n), (jl', s)] = delta(jl==jl') * D[k_s, m] *
     D[l_s, n] * GRAY[2].  Output [(f4, i), (jl, s)] is exactly the
     required DRAM order.
  5. One contiguous ~1.2MB store per octet.
"""

import numpy as np
from contextlib import ExitStack

import concourse.bass as bass
import concourse.mybir as mybir
from concourse import bacc, tile
from concourse.bass_utils import run_bass_kernel_spmd

_GRAY = np.array([0.2989, 0.587, 0.114], dtype=np.float32)
_BLK = 8
_B, _C, _T, _H, _W = 4, 3, 64, 224, 224
_NCORES = 8
_FPC = _T * _B // _NCORES  # 32 frames per core
_NI = _H // _BLK  # 28 block rows
_NJ = _W // _BLK  # 28 block cols
_NJQ = _NJ // 4  # 7 block quads
_F4 = 4  # frames per chunk
_P = _F4 * _NI  # 112 partitions
_FPO = 8  # frames per DMA octet
_NOCT = _FPC // _FPO  # 4 octets per core

_f32 = mybir.dt.float32
_GP_FD = 2048  # free-dim share of grayscale handled by GPSIMD (of 3584)


def _zigzag_idx(n=_BLK):
    idx = []
    for diag in range(2 * n - 1):
        if diag % 2 == 0:
            row = min(diag, n - 1); col = diag - row
            while row >= 0 and col < n:
                idx.append(row * n + col); row -= 1; col += 1
        else:
            col = min(diag, n - 1); row = diag - col
            while col >= 0 and row < n:
                idx.append(row * n + col); row += 1; col -= 1
    return np.array(idx, dtype=np.int64)


def _dct_mat(N=_BLK):
    n = np.arange(N, dtype=np.float64)
    k = np.arange(N, dtype=np.float64)[:, None]
    return 2.0 * np.cos(np.pi * (2.0 * n + 1.0) * k / (2.0 * N))


def _build_consts(S):
    keep = _zigzag_idx()[1:1 + S]
    D = _dct_mat()
    # Kron weights for m-pair mp: rows (m2, jl, n) = m2*64 + jl*8 + n with
    # m = 2*mp + m2; cols jl*S + s.  "Wide" = jl 0..7 (384 cols for S=48),
    # "narrow" = jl 0..3 padded with zero cols to 256 so fp32r streams at
    # 1 cycle/row.
    n_wide = 8 * S
    n_nar = max(4 * S, 256)
    Ww = [np.zeros((128, n_wide), dtype=np.float32) for _ in range(4)]
    Wn = [np.zeros((128, n_nar), dtype=np.float32) for _ in range(4)]
    for mp in range(4):
        for m2 in range(2):
            m = 2 * mp + m2
            for jl in range(8):
                for n in range(_BLK):
                    q = m2 * 64 + jl * 8 + n
                    for s in range(S):
                        k, l = divmod(int(keep[s]), _BLK)
                        v = np.float32(D[k, m] * D[l, n] * _GRAY[2])
                        Ww[mp][q, jl * S + s] = v
                        if jl < 4:
                            Wn[mp][q, jl * S + s] = v
    eye = np.eye(_P, dtype=np.float32)
    return np.concatenate(Ww, axis=1), np.concatenate(Wn, axis=1), eye


def build_nc(S, n_oct=_NOCT, n_rep=1, tr_mode=True, r_tr=True, r_kron=True,
             gp_fd=None, xbufs=2, gbufs=2):
    """Build and compile the per-core Bass program (n_oct * 8 frames).

    n_rep > 1 wraps the whole pipeline in a hardware loop that recomputes
    the same result n_rep times — used only for wall-clock timing.
    tr_mode: use the PE transpose instruction (transpose_mode) for the
    gather stage; r_tr / r_kron: run transposes / kron matmuls in float32r.
    """
    rdt = mybir.dt.float32r
    if gp_fd is None:
        gp_fd = _GP_FD
    Ww_np, Wn_np, eye_np = _build_consts(S)
    fpc = n_oct * _FPO
    n_wide = 8 * S
    n_nar = max(4 * S, 256)
    qlen = _NJ * S  # free extent of one chunk4 in the out tile
    # w-runs per half: (w offset, w length => j's covered); the last run
    # covers w 192:256 — 32 real cols + 32 junk cols (g8 is padded and the
    # narrow kron weights zero rows jl >= 4), keeping every transpose M=64
    wruns = [(0, 64), (64, 64), (128, 64), (192, 64)]

    w0 = float(_GRAY[0] / _GRAY[2])
    w1 = float(_GRAY[1] / _GRAY[2])

    nc = bacc.Bacc("TRN2", target_bir_lowering=False, debug=False)
    x_d = nc.dram_tensor("x", [_C, fpc, _H, _W], _f32, kind="ExternalInput")
    o_d = nc.dram_tensor("out", [fpc, _NI * _NJ * S], _f32, kind="ExternalOutput")
    I_d = nc.inline_tensor(eye_np, "I_const")
    Ww_d = nc.inline_tensor(Ww_np, "Ww_const")
    Wn_d = nc.inline_tensor(Wn_np, "Wn_const")

    ncopy = 0

    def copy_evict(dst, src):
        nonlocal ncopy
        if ncopy % 3 == 1:
            nc.vector.tensor_copy(dst, src)
        else:
            nc.scalar.copy(dst, src)
        ncopy += 1

    mult = mybir.AluOpType.mult
    add = mybir.AluOpType.add

    with tile.TileContext(nc) as tc, ExitStack() as ctx:
        cpool = ctx.enter_context(tc.tile_pool(name="const", bufs=1))
        xpool = ctx.enter_context(tc.tile_pool(name="xin", bufs=xbufs))
        gpool = ctx.enter_context(tc.tile_pool(name="gray", bufs=gbufs))
        t2pool = ctx.enter_context(tc.tile_pool(name="t2", bufs=2))
        opool = ctx.enter_context(tc.tile_pool(name="outp", bufs=2))
        pspool = ctx.enter_context(
            tc.tile_pool(name="ps", bufs=2, space=bass.MemorySpace.PSUM))

        I_sb = cpool.tile([_P, _P], _f32)
        Ww_sb = cpool.tile([128, 4 * n_wide], _f32)
        Wn_sb = cpool.tile([128, 4 * n_nar], _f32)
        nc.sync.dma_start(
            I_sb[:].bitcast(rdt) if r_tr else I_sb[:],
            I_d.ap().bitcast(rdt) if r_tr else I_d.ap())
        nc.sync.dma_start(
            Ww_sb[:].bitcast(rdt) if r_kron else Ww_sb[:],
            Ww_d.ap().bitcast(rdt) if r_kron else Ww_d.ap())
        nc.sync.dma_start(
            Wn_sb[:].bitcast(rdt) if r_kron else Wn_sb[:],
            Wn_d.ap().bitcast(rdt) if r_kron else Wn_d.ap())

        xap = x_d.ap()
        oap = o_d.ap()

        rep = ctx.enter_context(tc.For_i(0, n_rep, 1)) if n_rep > 1 else None
        del rep
        for o in range(n_oct):
            X = []
            for c in range(_C):
                xt = xpool.tile([_P, 2 * 1792], _f32, tag=f"x{c}")
                src = xap[c, o * _FPO:(o + 1) * _FPO].rearrange(
                    "(c2 f4) (i m) w -> (f4 i) c2 m w", c2=2, f4=_F4, m=_BLK)
                dst = xt.rearrange("p (c2 m w) -> p c2 m w", c2=2, m=_BLK)
                nc.sync.dma_start(dst, src)
                X.append(xt)
            # grayscale (third weight folded into Wp).  g8 is written in an
            # interleaved free layout c2*2048 + (w//64)*512 + m*64 + (w%64)
            # so each transpose's stationary chunk (an m-pair x 64-w-run) is
            # a contiguous 128-column slice.  DVE takes the fused 2-op path
            # on c2=0, GPSIMD the 4-op path on c2=1; both are 1x-mode ops so
            # the strided dst costs nothing extra.
            g8 = gpool.tile([_P, 2 * 2048], _f32, tag="g8")
            gw = (lambda ap: ap.bitcast(rdt)) if r_tr else (lambda ap: ap)
            # zero the pad: (c2, r=3, m, wr 32:64)
            pad = g8.rearrange("p (c2 r m wr) -> p c2 r m wr",
                               c2=2, r=4, m=_BLK)[:, :, 3, :, 32:]
            nc.vector.memset(pad.bitcast(mybir.dt.uint32), 0)
            gvw = g8.rearrange("p (c2 r m wr) -> p c2 m r wr",
                               c2=2, r=4, m=_BLK)
            def mw(ap):
                return ap.rearrange("p (m w) -> p m w", m=_BLK)

            MV = 3  # m-groups of c2=1 handled by DVE (rest on GPSIMD)
            glen = (8 - MV) * 224
            tg = gpool.tile([_P, glen], _f32, tag="gt", bufs=1)
            ug = gpool.tile([_P, glen], _f32, tag="gu", bufs=1)
            gsl = slice(1792 + MV * 224, 2 * 1792)
            nc.gpsimd.tensor_scalar_mul(tg[:], X[0][:, gsl], w0)
            nc.gpsimd.tensor_tensor(tg[:], tg[:], X[2][:, gsl], add)
            nc.gpsimd.tensor_scalar_mul(ug[:], X[1][:, gsl], w1)
            tgv = tg.rearrange("p (m w) -> p m w", m=_BLK - MV)
            ugv = ug.rearrange("p (m w) -> p m w", m=_BLK - MV)
            for c2 in range(2):
                sl = slice(c2 * 1792, (c2 + 1) * 1792)
                for r in range(4):
                    wlen = 64 if r < 3 else 32
                    ws = slice(r * 64, r * 64 + wlen)
                    dst = gvw[:, c2, :, r, 0:wlen]  # [112, 8, wlen]
                    mvh = _BLK if c2 == 0 else MV
                    dv = dst[:, 0:mvh]
                    nc.vector.scalar_tensor_tensor(
                        gw(dv), mw(X[0][:, sl])[:, 0:mvh, ws], w0,
                        mw(X[2][:, sl])[:, 0:mvh, ws], mult, add)
                    nc.vector.scalar_tensor_tensor(
                        gw(dv), mw(X[1][:, sl])[:, 0:mvh, ws], w1,
                        dv, mult, add)
                    if c2 == 1:
                        nc.gpsimd.tensor_tensor(
                            gw(dst[:, MV:]), tgv[:, :, ws], ugv[:, :, ws],
                            add)
            OUT8 = opool.tile([_P, 2 * qlen], _f32, tag="o8")
            for half in range(2):
                # T2sb[:, mp*448 + r*112 : +112] = (m2, jl, n) x (f4, i)
                T2sb = t2pool.tile([128, 4 * 448], _f32, tag="t2sb")
                for mp in range(4):
                    Pmp = pspool.tile([128, 448], _f32, tag="pst")
                    for r in range(4):
                        dst = Pmp[:, r * _P:(r + 1) * _P]
                        goff = half * 2048 + r * 512 + mp * 128
                        lhsT = g8[:, goff:goff + 128]
                        rhs = I_sb[:]
                        if r_tr:
                            dst = dst.bitcast(rdt)
                            lhsT = lhsT.bitcast(rdt)
                            rhs = rhs.bitcast(rdt)
                        if tr_mode:
                            nc.tensor.transpose(dst, lhsT, rhs)
                        else:
                            nc.tensor.matmul(dst, lhsT, rhs,
                                             start=True, stop=True)
                    edst = T2sb[:, mp * 448:(mp + 1) * 448]
                    esrc = Pmp[:]
                    if r_kron:
                        edst = edst.bitcast(rdt)
                        esrc = esrc.bitcast(rdt)
                    copy_evict(edst, esrc)
                for r in range(4):
                    wide = r < 3
                    N = n_wide if wide else n_nar
                    Ops = pspool.tile([_P, n_wide], _f32, tag="ops")
                    for mp in range(4):
                        dst = Ops[:, :N]
                        lhsT = T2sb[:, mp * 448 + r * _P:mp * 448 + (r + 1) * _P]
                        rhs = (Ww_sb[:, mp * n_wide:(mp + 1) * n_wide] if wide
                               else Wn_sb[:, mp * n_nar:(mp + 1) * n_nar])
                        if r_kron:
                            # out stays f32 (matmul requires fp32 PSUM out)
                            lhsT = lhsT.bitcast(rdt)
                            rhs = rhs.bitcast(rdt)
                        nc.tensor.matmul(dst, lhsT, rhs,
                                         start=(mp == 0), stop=(mp == 3))
                    ecols = n_wide if wide else 4 * S
                    copy_evict(
                        OUT8[:, half * qlen + r * n_wide:
                             half * qlen + r * n_wide + ecols],
                        Ops[:, :ecols])
            dsto = oap[o * _FPO:(o + 1) * _FPO].rearrange(
                "(c2 f4) (i q) -> (f4 i) c2 q", c2=2, i=_NI)
            nc.sync.dma_start(dsto, OUT8.rearrange("p (c2 q) -> p c2 q", c2=2))

    nc.compile()
    return nc


def _shard_inputs(x):
    in_maps = []
    for k in range(_NCORES):
        b, th = k // 2, k % 2
        xs = np.ascontiguousarray(
            x[b, :, th * _FPC:(th + 1) * _FPC], dtype=np.float32)
        in_maps.append({"x": xs})
    return in_maps


def kernel(x, remove_last_ac):
    x = np.asarray(x)
    r = int(remove_last_ac)
    S = _BLK * _BLK - 1 - r
    nc = build_nc(S)
    res = run_bass_kernel_spmd(nc, _shard_inputs(x), list(range(_NCORES)))
    out = np.empty((_B, _T, _NI * _NJ * S), dtype=np.float32)
    for k in range(_NCORES):
        b, th = k // 2, k % 2
        out[b, th * _FPC:(th + 1) * _FPC] = res.results[k]["out"]
    return out
